# revision 1
# baseline (speedup 1.0000x reference)
"""Trainium2 Bass kernel for nn_LossFunction_12532714569881.

Computes, for x: [N=8192, 2, D=256] fp32, w, b scalars:
    P = x[:,0,:]; A = x[:,1,:]
    logits = (P @ A^T) / max(|p_i||a_j|, eps) * w + b        # [N, N]
    loss = -mean_i(log_softmax(logits)[i, i])

Strategy (8 NeuronCores, SPMD, single launch):
  - Row-shard the NxN logits: core c owns rows R=c*1024 .. R+1024.
  - Every core receives its positive block (xp), its anchor block (xad)
    and the FULL anchor matrix (xa).
  - Anchors: per-1/4 column group, sum-of-squares on DVE (fp32), 1/norm
    via exp(-0.5*ln(s)) on ACT (single activation table set, see
    _patch_act_tables), fused normalize+bf16-cast on GpSimd, then
    transposed into AnT via the DMA xbar transpose (2-byte path) --
    the tensor engine never touches transposes.
  - Positives stay raw: the per-row scale w/|p_i| folds into the exp
    activation's per-partition scale operand.
  - Main loop pipelines per column group: bf16 matmuls (1 cyc/row)
    accumulate K=256 in two 128-chunks into [128, 2048] PSUM tiles
    (4 banks x 2 bufs = all 8 banks); the scalar engine applies
    exp(scale_i * dot - |w|) with a fused row-sum (accum_out).
  - Since cos in [-1,1], logits <= |w|+b, so the constant shift |w|+b
    replaces the row-max pass of a standard softmax (no overflow).
  - The diagonal logit (the label term) is recomputed exactly in fp32
    on the vector engine from the raw blocks, so the bf16 matmul noise
    only perturbs the log-sum-exp, where it averages out.
  - Each core emits one partial scalar = sum of its 1024 row losses
    (row loss = ln(S_i) + |w| - w*cos_ii); the host sums 8 partials,
    divides by N.

kernel(**inputs) -> np.float32 scalar (shape () like the reference).
"""

import numpy as np

N = 8192
D = 256
NCORES = 8
RPC = N // NCORES          # 1024 rows per core
P = 128                    # partitions
NT_A = N // P              # 64 anchor tiles
NT_P = RPC // P            # 8 positive tiles / m-chunks
KH = D // P                # 2 k-halves
NB = 512                   # matmul free-dim per instruction (1 psum bank)
GCOLS = 2048               # columns per activation / column group
NGRP = N // GCOLS          # 4 column groups
TPG = GCOLS // P           # 16 anchor tiles per column group
EPS = 1e-8                 # reference eps (negligible for randn rows)

# knob: bfloat16 (1 cyc/row PE) or float32 (2-pass, ~2x slower, exact)
MM_DTYPE = "bfloat16"

_BUILD_CACHE = {}
_ACT_TABLES_PATCHED = False
_LDW_OPT_PATCHED = False
LDW_OPT = False


def _patch_ldw_opt():
    """walrus's redundant-LDWEIGHTS elision is hardcoded off in
    bass_utils; consecutive same-weight matmuls (our k-runs of 4) then
    re-load the PE array every instruction.  Rewrite the flag on the
    walrus command line.  Validated against the reference output."""
    global _LDW_OPT_PATCHED
    if _LDW_OPT_PATCHED or not LDW_OPT:
        return
    import concourse.bass_utils as bu

    orig_run = bu.run_command

    def patched(argv, **kwargs):
        argv = [a.replace("--enable-ldw-opt=false", "--enable-ldw-opt=true")
                if isinstance(a, str) else a for a in argv]
        return orig_run(argv, **kwargs)

    bu.run_command = patched
    _LDW_OPT_PATCHED = True


def _patch_act_tables():
    """Make both Exp and Ln resolve to the one table set that contains
    them both (natural_log_exp_and_others), so the kernel needs a single
    ACT_TABLE_LOAD instead of thrashing between exp/ln sets.  Set ids
    are positional, so we filter set contents rather than reorder."""
    global _ACT_TABLES_PATCHED
    if _ACT_TABLES_PATCHED:
        return
    import concourse.bacc as bacc_mod
    import concourse.bass_interp as interp_mod
    import concourse.mybir as mybir
    from concourse import hw_specs

    AF = mybir.ActivationFunctionType
    orig = hw_specs.get_activation_tables

    def patched(module_arch):
        tabs = orig(module_arch)
        out = {}
        for name, funcs in tabs.items():
            f = set(funcs)
            if name != "natural_log_exp_and_others":
                f.discard(AF.Exp)
                f.discard(AF.Ln)
            out[name] = f
        return out

    bacc_mod.get_activation_tables = patched
    interp_mod.get_activation_tables = patched
    _ACT_TABLES_PATCHED = True


def _build(w: float, b: float, mm_dtype: str):
    from contextlib import ExitStack

    import concourse.bass as bass  # noqa: F401
    import concourse.mybir as mybir
    import concourse.tile as tile
    from concourse import bacc

    _patch_act_tables()
    _patch_ldw_opt()

    f32 = mybir.dt.float32
    mm_dt = getattr(mybir.dt, mm_dtype)
    AF = mybir.ActivationFunctionType
    ALU = mybir.AluOpType
    AX = mybir.AxisListType

    absw = abs(float(w))
    bias_exp = -absw          # exp(scale_i * dot + b - shift), shift = |w| + b

    nc = bacc.Bacc("TRN2", target_bir_lowering=False, debug=False)

    xp = nc.dram_tensor("xp", [RPC, D], f32, kind="ExternalInput").ap()
    xad = nc.dram_tensor("xad", [RPC, D], f32, kind="ExternalInput").ap()
    xa = nc.dram_tensor("xa", [N, D], f32, kind="ExternalInput").ap()
    out_partial = nc.dram_tensor("partial", [1, 1], f32, kind="ExternalOutput").ap()
    out_rowloss = nc.dram_tensor("rowloss", [P, NT_P], f32, kind="ExternalOutput").ap()

    with tile.TileContext(nc) as tc:
        with ExitStack() as ctx:
            sing = ctx.enter_context(tc.tile_pool(name="sing", bufs=1))
            sq_pool = ctx.enter_context(tc.tile_pool(name="sqp", bufs=3))
            exp_pool = ctx.enter_context(tc.tile_pool(name="expp", bufs=3))

            # ---- persistent SBUF tensors (split per group for fine deps)
            xa_raw = [sing.tile([P, TPG * D], f32, tag=f"xar{g}", name=f"xar{g}")
                      for g in range(NGRP)]
            xa_bf = [sing.tile([P, TPG * D], mm_dt, tag=f"xab{g}", name=f"xab{g}")
                     for g in range(NGRP)]
            ssq_a = [sing.tile([P, TPG], f32, tag=f"ssqa{g}", name=f"ssqa{g}")
                     for g in range(NGRP)]
            lns_a = [sing.tile([P, TPG], f32, tag=f"lnsa{g}", name=f"lnsa{g}")
                     for g in range(NGRP)]
            inv_a = [sing.tile([P, TPG], f32, tag=f"inva{g}", name=f"inva{g}")
                     for g in range(NGRP)]
            ant = [[sing.tile([P, GCOLS], mm_dt, tag=f"ant{h}_{g}",
                              name=f"ant{h}_{g}") for g in range(NGRP)]
                   for h in range(KH)]

            sb_xp = sing.tile([P, NT_P * D], f32, tag="xp")     # positives raw
            sb_xad = sing.tile([P, NT_P * D], f32, tag="xad")   # own anchors raw
            sb_xp_bf = sing.tile([P, NT_P * D], mm_dt, tag="xpbf")
            pnt = [sing.tile([P, RPC], mm_dt, tag=f"pnt{h}", name=f"pnt{h}")
                   for h in range(KH)]
            ident = sing.tile([P, P], mm_dt, tag="ident")
            ones = sing.tile([P, 1], f32, tag="ones")
            bias_t = sing.tile([P, 1], f32, tag="bias_t")

            ssq_pd = sing.tile([P, 2 * NT_P], f32, tag="ssqpd")  # P | XAD
            lns_pd = sing.tile([P, 2 * NT_P], f32, tag="lnspd")
            inv_pd = sing.tile([P, 2 * NT_P], f32, tag="invpd")
            winvp = sing.tile([P, NT_P], f32, tag="winvp")       # w / |p_i|
            pa = sing.tile([P, NT_P], f32, tag="pa")             # dot(p_i,a_i)
            ssum = sing.tile([P, NT_P * NGRP], f32, tag="ssum")
            srow = sing.tile([P, NT_P], f32, tag="srow")
            lnS = sing.tile([P, NT_P], f32, tag="lnS")
            cosd = sing.tile([P, NT_P], f32, tag="cosd")
            rowloss = sing.tile([P, NT_P], f32, tag="rowloss")
            rsum = sing.tile([P, 1], f32, tag="rsum")
            sc_out = sing.tile([1, 1], f32, tag="sc_out")

            invad = inv_pd[:, NT_P:2 * NT_P]

            from concourse.masks import make_identity
            make_identity(nc, ident[:])
            nc.vector.memset(ones, 1.0)
            nc.vector.memset(bias_t, bias_exp)

            # ---- loads: spread issue across engines (sync issue is ~3us
            # per big DMA and would serialize the group loads)
            nc.scalar.dma_start(
                out=sb_xp.rearrange("p (t d) -> p t d", d=D),
                in_=xp.rearrange("(t p) d -> p t d", p=P),
            )
            nc.gpsimd.dma_start(
                out=sb_xad.rearrange("p (t d) -> p t d", d=D),
                in_=xad.rearrange("(t p) d -> p t d", p=P),
            )
            load_eng = [nc.sync, nc.scalar, nc.sync, nc.gpsimd]
            for g in range(NGRP):
                load_eng[g].dma_start(
                    out=xa_raw[g].rearrange("p (t d) -> p t d", d=D),
                    in_=xa.rearrange("(g t p) d -> p g t d", p=P, t=TPG)[
                        :, g, :, :],
                )

            # ---- P-side prep (fp32-exact stats for the diagonal) -------
            def sumsq_f32(src, t, acc, col):
                scr = sq_pool.tile([P, D], f32, tag="sqscr", name="sqscr")
                nc.vector.scalar_tensor_tensor(
                    out=scr,
                    in0=src[:, t * D:(t + 1) * D],
                    scalar=1.0,
                    in1=src[:, t * D:(t + 1) * D],
                    op0=ALU.mult,
                    op1=ALU.mult,
                    accum_out=acc[:, col:col + 1],
                )

            # winvp chain first (gates the first exp): cast on DVE, P sumsq
            # on the scalar engine (Square+accum; it idles this early)
            def sumsq_act(src, t, acc, col):
                scr = sq_pool.tile([P, D], f32, tag="asqscr", name="asqscr")
                nc.scalar.activation(
                    scr, src[:, t * D:(t + 1) * D], AF.Square,
                    accum_out=acc[:, col:col + 1],
                )

            for half in range(2):
                nc.vector.tensor_copy(
                    sb_xp_bf[:, half * 4 * D:(half + 1) * 4 * D],
                    sb_xp[:, half * 4 * D:(half + 1) * 4 * D],
                )
            for t in range(NT_P):
                sumsq_act(sb_xp, t, ssq_pd, t)

            nc.scalar.activation(lns_pd[:, 0:NT_P], ssq_pd[:, 0:NT_P], AF.Ln)
            nc.scalar.activation(inv_pd[:, 0:NT_P], lns_pd[:, 0:NT_P],
                                 AF.Exp, scale=-0.5)
            nc.vector.tensor_scalar_mul(winvp, inv_pd[:, 0:NT_P], float(w))

            # ---- per column group: norms -> normalize -> transpose -> mm
            # PE transposes batch 16 [128,128] bf16 tiles into one PSUM
            # claim; transpose claims share the matmul pool slots (same
            # tag + byte size) so 2x4 banks covers everything.
            with tc.tile_pool(name="psM", bufs=2, space="PSUM") as psM:
                def transpose_batch(src_bf, dst, h, ntile):
                    ps = psM.tile([P, ntile * P], mm_dt, tag="psmm",
                                  name="pst")
                    for q in range(ntile):
                        nc.tensor.transpose(
                            ps[:, q * P:(q + 1) * P],
                            src_bf[:, q * D + h * P: q * D + (h + 1) * P],
                            ident,
                        )
                    nc.vector.tensor_copy(dst, ps)

                # positive transposes first (small, needed by every group)
                for h in range(KH):
                    transpose_batch(sb_xp_bf, pnt[h][:, :], h, NT_P)

                for g in range(NGRP):
                    for t in range(TPG):
                        # group 0 norms on ACT (idle early); rest on DVE
                        if g == 0:
                            sumsq_act(xa_raw[g], t, ssq_a[g], t)
                            continue
                        scr = sq_pool.tile([P, D], f32, tag="sqscr",
                                           name="sqscr")
                        nc.vector.scalar_tensor_tensor(
                            out=scr,
                            in0=xa_raw[g][:, t * D:(t + 1) * D],
                            scalar=1.0,
                            in1=xa_raw[g][:, t * D:(t + 1) * D],
                            op0=ALU.mult,
                            op1=ALU.mult,
                            accum_out=ssq_a[g][:, t:t + 1],
                        )
                    nc.scalar.activation(lns_a[g], ssq_a[g], AF.Ln)
                    nc.scalar.activation(inv_a[g], lns_a[g], AF.Exp,
                                         scale=-0.5)
                    # fused normalize + bf16 cast on DVE
                    for t in range(TPG):
                        nc.vector.tensor_scalar_mul(
                            xa_bf[g][:, t * D:(t + 1) * D],
                            xa_raw[g][:, t * D:(t + 1) * D],
                            inv_a[g][:, t:t + 1],
                        )
                    # anchor transposes on the tensor engine
                    for h in range(KH):
                        transpose_batch(xa_bf[g], ant[h][g][:, :], h, TPG)
                    # matmul + exp sweep over all row chunks for this group
                    for m in range(NT_P):
                        ps = psM.tile([P, GCOLS], f32, tag="psmm", name="psmm")
                        for h in range(KH):
                            for nn in range(GCOLS // NB):
                                nc.tensor.matmul(
                                    ps[:, nn * NB:(nn + 1) * NB],
                                    pnt[h][:, m * P:(m + 1) * P],
                                    ant[h][g][:, nn * NB:(nn + 1) * NB],
                                    start=(h == 0),
                                    stop=(h == KH - 1),
                                )
                        scr = exp_pool.tile([P, GCOLS], f32, tag="expscr",
                                            name="expscr")
                        nc.scalar.activation(
                            scr,
                            ps,
                            AF.Exp,
                            bias=bias_t[:, 0:1],
                            scale=winvp[:, m:m + 1],
                            accum_out=ssum[:, m * NGRP + g: m * NGRP + g + 1],
                        )

            # ---- tail-only stats (emitted late; overlap the main loop) -
            for t in range(NT_P):
                sumsq_f32(sb_xad, t, ssq_pd, NT_P + t)
            for t in range(NT_P):
                scr = sq_pool.tile([P, D], f32, tag="sqscr", name="sqscr")
                nc.vector.scalar_tensor_tensor(
                    out=scr,
                    in0=sb_xp[:, t * D:(t + 1) * D],
                    scalar=1.0,
                    in1=sb_xad[:, t * D:(t + 1) * D],
                    op0=ALU.mult,
                    op1=ALU.mult,
                    accum_out=pa[:, t:t + 1],
                )
            nc.scalar.activation(lns_pd[:, NT_P:], ssq_pd[:, NT_P:], AF.Ln)
            nc.scalar.activation(inv_pd[:, NT_P:], lns_pd[:, NT_P:],
                                 AF.Exp, scale=-0.5)

            # ---- tail --------------------------------------------------
            nc.vector.tensor_reduce(
                srow,
                ssum.rearrange("p (m g) -> p m g", g=NGRP),
                axis=AX.X,
                op=ALU.add,
            )
            nc.scalar.activation(lnS, srow, AF.Ln)
            # rowloss = lnS + |w| - winvp*invad*pa
            nc.vector.tensor_mul(cosd, pa, invad)
            nc.vector.tensor_mul(cosd, cosd, winvp)   # = w * cos_ii
            nc.vector.scalar_tensor_tensor(
                out=rowloss,
                in0=cosd,
                scalar=-1.0,
                in1=lnS,
                op0=ALU.mult,
                op1=ALU.add,
            )
            nc.vector.tensor_scalar_add(rowloss, rowloss, absw)
            nc.vector.reduce_sum(rsum, rowloss, axis=AX.X)
            nc.sync.dma_start(out=out_rowloss, in_=rowloss)

            with tc.tile_pool(name="psF", bufs=1, space="PSUM") as psF:
                pfin = psF.tile([1, 1], f32, tag="pfin")
                nc.tensor.matmul(pfin, rsum, ones, start=True, stop=True)
                nc.vector.tensor_copy(sc_out, pfin)
            nc.sync.dma_start(out=out_partial, in_=sc_out)

    nc.compile()
    return nc


def _get_nc(w: float, b: float):
    key = (float(w), float(b), MM_DTYPE)
    if key not in _BUILD_CACHE:
        _BUILD_CACHE[key] = _build(float(w), float(b), MM_DTYPE)
    return _BUILD_CACHE[key]


def kernel(x, w, b, epoch=None, **_unused):
    from concourse.bass_utils import run_bass_kernel_spmd

    x = np.asarray(x, dtype=np.float32)
    w_f = float(np.asarray(w))
    b_f = float(np.asarray(b))
    assert x.shape == (N, 2, D), x.shape

    nc = _get_nc(w_f, b_f)

    xa_full = np.ascontiguousarray(x[:, 1, :])
    in_maps = []
    for c in range(NCORES):
        r0 = c * RPC
        in_maps.append({
            "xp": np.ascontiguousarray(x[r0:r0 + RPC, 0, :]),
            "xad": np.ascontiguousarray(x[r0:r0 + RPC, 1, :]),
            "xa": xa_full,
        })

    res = run_bass_kernel_spmd(nc, in_maps, list(range(NCORES)))
    total = 0.0
    for c in range(NCORES):
        total += float(res.results[c]["partial"][0, 0])
    loss = total / N
    return np.float32(loss)

